# revision 17
# baseline (speedup 1.0000x reference)
"""GCNConv forward on Trainium2, data-parallel over batch across 8 NeuronCores.

Computes, per batch element:
    A    = binarize(adj) with forced self-loops (diag = 1)
    deg  = A.sum(-1);  dinv = rsqrt(deg)
    out  = tanh( Dinv @ A @ Dinv @ (x @ W) + b )

Layout strategy per core (one [N,N] graph per core, N=2048, F=O=512):
  - x chunks are cast (DVE) to the matmul dtype, PE-transposed into x^T tiles
    (stationary operand for h = x @ W).
  - adj rows are streamed, binarized on DVE (with fused row-sum for deg via
    accum_out), diagonal forced via a scalar_tensor_tensor with an identity
    tile, then PE-transposed into A^T tiles (exact 0/1 values).
  - agg matmul contracts A^T tiles (stationary) against u = Dinv @ h (moving).
  - epilogue fuses dinv row-scale + bias add on DVE, tanh on ACT.

Hardware constraint honored throughout: a PE (matmul/transpose) instruction
can carry only ONE semaphore wait, so every tile consumed by PE and every
PSUM slot reused by PE is produced/released by DVE only.

Modes (matmul dtypes; A entries are exact 0/1 in every mode):
  fp16:    A^T, x, W, u rounded to fp16.  1 cyc/row matmuls.   ~4e-4 rel err.
  split16: like fp16 but u/x/W are split into fp16 hi+lo pairs; agg runs 2
           matmuls per tile and h runs 3.                      ~1e-6 rel err.
  f32r:    operands rounded to fp32r; A^T expanded per-row-block from a
           resident fp16 A (SBUF budget).  1 cyc/row matmuls.
  fp32:    exact fp32 everywhere.  4 cyc/row matmuls (slow, reference).
"""

import os
import sys
from contextlib import ExitStack

import numpy as np

for _p in (
    "/root/.axon_site",
    "/root/.axon_site/_ro/trn_rl_repo",
    "/root/.axon_site/_ro/pypackages",
    "/opt/trn_rl_repo",
):
    if os.path.isdir(_p) and _p not in sys.path:
        sys.path.append(_p)

import concourse.bacc as bacc
import concourse.bass as bass
import concourse.mybir as mybir
import concourse.tile as tile
from concourse.bass_utils import run_bass_kernel_spmd

# Problem sizes (hardcoded per harness contract).
B, N, F_IN, O = 8, 2048, 512, 512
P = 128
NCH = N // P  # 16 node chunks
FCH = F_IN // P  # 4 feature chunks

F32 = mybir.dt.float32
F32R = mybir.dt.float32r
F16 = mybir.dt.float16
AF = mybir.ActivationFunctionType
OP = mybir.AluOpType

MODE = os.environ.get("GCN_MODE", "split16")


def build_nc(mode: str) -> bacc.Bacc:
    resident = mode in ("fp16", "split16")  # A^T kept fully expanded in SBUF
    a_dt = F16  # storage dtype of binarized A (exact 0/1)
    mm_dt = {"fp16": F16, "split16": F16, "f32r": F32R, "fp32": F32}[mode]
    split = mode == "split16"

    nc = bacc.Bacc("TRN2", target_bir_lowering=False, debug=False)
    x_d = nc.dram_tensor("inputs", [N, F_IN], F32, kind="ExternalInput").ap()
    adj_d = nc.dram_tensor("adj", [N, N], F32, kind="ExternalInput").ap()
    w_d = nc.dram_tensor("W", [F_IN, O], F32, kind="ExternalInput").ap()
    bb_d = nc.dram_tensor("bb", [P, O], F32, kind="ExternalInput").ap()
    eyea_d = nc.dram_tensor("eye_a", [P, P], F16, kind="ExternalInput").ap()
    eye32_d = nc.dram_tensor("eye32", [P, P], F32, kind="ExternalInput").ap()
    out_d = nc.dram_tensor("out", [N, O], F32, kind="ExternalOutput").ap()

    with tile.TileContext(nc) as tc, ExitStack() as ctx:
        ep = ctx.enter_context
        const = ep(tc.tile_pool(name="const", bufs=1))
        big = ep(tc.tile_pool(name="big", bufs=1))
        acp = ep(tc.tile_pool(name="ac", bufs=2))
        if resident:
            anp = ep(tc.tile_pool(name="an", bufs=2))
        else:
            atbp = ep(tc.tile_pool(name="atb", bufs=2))
        xcp = ep(tc.tile_pool(name="xc", bufs=2))
        xdp = ep(tc.tile_pool(name="xd", bufs=2))
        sp = ep(tc.tile_pool(name="s", bufs=4))
        dp = ep(tc.tile_pool(name="delta", bufs=1 if split else 2))
        ntp = ep(tc.tile_pool(name="nt", bufs=2))
        t1p = ep(tc.tile_pool(name="t1", bufs=2))
        tpp = ep(tc.tile_pool(name="tp", bufs=3, space=bass.MemorySpace.PSUM))
        mmp = ep(tc.tile_pool(name="mm", bufs=2, space=bass.MemorySpace.PSUM))

        # ---- constants.  PE-consumed tiles must be DVE-produced: identity
        # matrices and W are re-rounded through DVE copies.
        eye_raw = const.tile([P, P], F16)
        nc.sync.dma_start(eye_raw[:], eyea_d)
        eye32_raw = const.tile([P, P], F32)
        nc.sync.dma_start(eye32_raw[:], eye32_d)
        eye_a = const.tile([P, P], a_dt)
        nc.vector.tensor_copy(eye_a[:], eye_raw[:])
        if mm_dt == F16:
            eye_mm = eye_a
        else:
            eye_mm = const.tile([P, P], mm_dt)
            nc.vector.tensor_copy(eye_mm[:], eye32_raw[:])
        bb = const.tile([P, O], F32)
        nc.sync.dma_start(bb[:], bb_d)
        # stage fp32 W in a single-buffer slot, keep only the DVE-rounded copy
        w32 = acp.tile([P, N], F32, tag="ac", name="w32stage", bufs=1)
        w32v = w32[:].rearrange("p (fc o) -> p fc o", fc=FCH)
        nc.sync.dma_start(w32v, w_d.rearrange("(fc p) o -> p fc o", p=P))
        w_mm = const.tile([P, FCH, O], mm_dt)
        nc.vector.tensor_copy(w_mm[:].rearrange("p fc o -> p (fc o)"), w32[:])
        if split:
            w_lo = const.tile([P, FCH, O], F16)
            nc.vector.scalar_tensor_tensor(
                w_lo[:].rearrange("p fc o -> p (fc o)"),
                w32[:],
                0.0,
                w_mm[:].rearrange("p fc o -> p (fc o)"),
                OP.bypass,
                OP.subtract,
            )
        deg = const.tile([P, NCH], F32)
        dinv = const.tile([P, NCH], F32)

        # ---- persistent state
        if resident:
            at = big.tile([P, NCH, N], a_dt)  # at[p, jc, i] = A[i, jc*P+p]
        else:
            anat = big.tile([P, NCH, N], a_dt)  # anat[p, r, j] = A[r*P+p, j]
        xt = big.tile([P, FCH, N], mm_dt)  # xt[p, fc, i] = x[i, fc*P+p]
        if split:
            xt_lo = big.tile([P, FCH, N], F16)
        if resident:
            h_sb = big.tile([P, NCH, O], F32)  # h = x @ W (unscaled)
        u = big.tile([P, NCH, O], mm_dt)  # u = dinv * h
        if split:
            u_lo = big.tile([P, NCH, O], F16)

        # ---- x path: load, DVE-cast to matmul dtype, PE transpose
        for ic in range(NCH):
            xc = xcp.tile([P, F_IN], F32, tag="xc", name=f"xc{ic}")
            nc.sync.dma_start(xc[:], x_d[ic * P : (ic + 1) * P, :])
            xch = xdp.tile([P, F_IN], mm_dt, tag="xch", name=f"xch{ic}")
            nc.vector.tensor_copy(xch[:], xc[:])
            srcs = [(xch, xt)]
            if split:
                xcl = xdp.tile([P, F_IN], F16, tag="xcl", name=f"xcl{ic}")
                nc.vector.scalar_tensor_tensor(
                    xcl[:], xc[:], 0.0, xch[:], OP.bypass, OP.subtract
                )
                srcs.append((xcl, xt_lo))
            for src, dst_all in srcs:
                xp = tpp.tile([P, 512], mm_dt, tag="tp", name=f"xp{ic}")
                for fc in range(FCH):
                    nc.tensor.transpose(
                        xp[:, fc * P : (fc + 1) * P],
                        src[:, fc * P : (fc + 1) * P],
                        eye_mm[:] if src is xch else eye_a[:],
                    )
                nc.scalar.copy(
                    dst_all[:, :, ic * P : (ic + 1) * P],
                    xp[:].rearrange("p (fc i) -> p fc i", fc=FCH),
                )

        # ---- mm1: h = x @ W
        def mm1(ic):
            hp = mmp.tile([P, O], F32, tag="mm", name=f"hp{ic}")
            pairs = [(xt, w_mm)]
            if split:
                pairs += [(xt_lo, w_mm), (xt, w_lo)]
            n_mm = len(pairs) * FCH
            k = 0
            for lhs_t, rhs_w in pairs:
                for fc in range(FCH):
                    nc.tensor.matmul(
                        hp[:],
                        lhs_t[:, fc, ic * P : (ic + 1) * P],
                        rhs_w[:, fc, :],
                        start=(k == 0),
                        stop=(k == n_mm - 1),
                    )
                    k += 1
            return hp

        if resident:
            for ic in range(NCH):
                hp = mm1(ic)
                nc.scalar.copy(h_sb[:, ic, :], hp[:])

        # ---- adj path: binarize + self-loops + deg (+ transpose into A^T)
        for r in range(NCH):
            # SWDGE cast-load: adj fp32 in DRAM -> fp16 in SBUF.  Safe for
            # binarization: any nonzero fp32 from real data stays nonzero.
            ac = acp.tile([P, N], F16, tag="ach", name=f"ac{r}")
            nc.gpsimd.dma_start(ac[:], adj_d[r * P : (r + 1) * P, :])
            s1 = sp.tile([P, 1], F32, tag="s", name=f"s1_{r}")
            s2 = sp.tile([P, 1], F32, tag="s", name=f"s2_{r}")
            if resident:
                an = anp.tile([P, N], a_dt, tag="an", name=f"an{r}")[:]
            else:
                an = anat[:, r, :]
            rsl = slice(r * P, (r + 1) * P)
            nc.vector.tensor_scalar(
                an, ac[:], 0.0, None, OP.not_equal, OP.add, accum_out=s1[:]
            )
            delta = dp.tile([P, P], a_dt, tag="delta", name=f"delta{r}")
            nc.vector.scalar_tensor_tensor(
                delta[:],
                ac[:, rsl],
                0.0,
                eye_a[:],
                OP.not_equal,
                OP.is_lt,
                accum_out=s2[:],
            )
            nc.vector.tensor_add(an[:, rsl], an[:, rsl], delta[:])
            nc.vector.tensor_add(deg[:, r : r + 1], s1[:], s2[:])
            if resident:
                for g in range(4):
                    tp = tpp.tile([P, 512], a_dt, tag="tp", name=f"tpA{r}_{g}")
                    for k in range(4):
                        c = 4 * g + k
                        nc.tensor.transpose(
                            tp[:, k * P : (k + 1) * P],
                            an[:, c * P : (c + 1) * P],
                            eye_a[:],
                        )
                    nc.vector.tensor_copy(
                        at[:, 4 * g : 4 * g + 4, r * P : (r + 1) * P],
                        tp[:].rearrange("p (c i) -> p c i", c=4),
                    )

        # ---- dinv = rsqrt(deg), deg >= 1 always (self-loops)
        rec = ntp.tile([P, NCH], F32, tag="nt")
        nc.vector.reciprocal(rec[:], deg[:])
        nc.scalar.sqrt(dinv[:], rec[:])
        for _ in range(2):  # Newton: y <- y * (1.5 - 0.5 * d * y^2)
            t = ntp.tile([P, NCH], F32, tag="nt")
            nc.vector.tensor_mul(t[:], dinv[:], dinv[:])
            nc.vector.tensor_mul(t[:], t[:], deg[:])
            nc.vector.tensor_scalar(t[:], t[:], -0.5, 1.5, OP.mult, OP.add)
            nc.vector.tensor_mul(dinv[:], dinv[:], t[:])

        # ---- u = dinv * h
        if resident:
            for jc in range(NCH):
                nc.vector.tensor_scalar(
                    u[:, jc, :], h_sb[:, jc, :], dinv[:, jc : jc + 1], None, OP.mult
                )
                if split:
                    nc.vector.scalar_tensor_tensor(
                        u_lo[:, jc, :],
                        h_sb[:, jc, :],
                        dinv[:, jc : jc + 1],
                        u[:, jc, :],
                        OP.mult,
                        OP.subtract,
                    )
        else:
            for jc in range(NCH):
                hp = mm1(jc)
                nc.vector.tensor_scalar(
                    u[:, jc, :], hp[:], dinv[:, jc : jc + 1], None, OP.mult
                )

        # ---- agg: v = A @ u, then out = tanh(dinv * v + b)
        u_list = [u, u_lo] if split else [u]
        for ic in range(NCH):
            if resident:
                lhs = at[:, :, ic * P : (ic + 1) * P]  # [P, jc, i]
            else:
                # expand A^T row-block for this i-chunk from resident fp16 A
                atb = atbp.tile([P, NCH, P], mm_dt, tag="atb", name=f"atb{ic}")
                for g in range(4):
                    tp = tpp.tile([P, 512], a_dt, tag="tp", name=f"tpB{ic}_{g}")
                    for k in range(4):
                        c = 4 * g + k
                        nc.tensor.transpose(
                            tp[:, k * P : (k + 1) * P],
                            anat[:, ic, c * P : (c + 1) * P],
                            eye_a[:],
                        )
                    nc.vector.tensor_copy(
                        atb[:, 4 * g : 4 * g + 4, :],
                        tp[:].rearrange("p (c i) -> p c i", c=4),
                    )
                lhs = atb[:]
            vp = mmp.tile([P, O], F32, tag="mm", name=f"vp{ic}")
            n_mm = NCH * len(u_list)
            k = 0
            for jc in range(NCH):
                for uu in u_list:
                    nc.tensor.matmul(
                        vp[:],
                        lhs[:, jc, :],
                        uu[:, jc, :],
                        start=(k == 0),
                        stop=(k == n_mm - 1),
                    )
                    k += 1
            t1 = t1p.tile([P, O], F32, tag="t1", name=f"t1_{ic}")
            nc.vector.scalar_tensor_tensor(
                t1[:], vp[:], dinv[:, ic : ic + 1], bb[:], OP.mult, OP.add
            )
            nc.scalar.activation(t1[:], t1[:], AF.Tanh)
            nc.sync.dma_start(out_d[ic * P : (ic + 1) * P, :], t1[:])

    nc.compile()
    return nc


_NC_CACHE: dict[str, bacc.Bacc] = {}


def get_nc(mode: str = MODE) -> bacc.Bacc:
    if mode not in _NC_CACHE:
        nc = build_nc(mode)
        # strip sim-only trap/callback instructions, matching run_on_hw_raw
        from concourse.bass_interp import get_hw_module

        nc.m = get_hw_module(nc.m)
        _NC_CACHE[mode] = nc
    return _NC_CACHE[mode]


def make_in_maps(inputs, adj, W, b):
    bb = np.ascontiguousarray(np.broadcast_to(b.astype(np.float32), (P, O)))
    eye_a = np.eye(P, dtype=np.float16)
    eye32 = np.eye(P, dtype=np.float32)
    return [
        {
            "inputs": np.ascontiguousarray(inputs[k], dtype=np.float32),
            "adj": np.ascontiguousarray(adj[k], dtype=np.float32),
            "W": np.ascontiguousarray(W, dtype=np.float32),
            "bb": bb,
            "eye_a": eye_a,
            "eye32": eye32,
        }
        for k in range(B)
    ]


def kernel(**inputs) -> np.ndarray:
    x, adj = inputs["inputs"], inputs["adj"]
    W, b = inputs["W"], inputs["b"]
    assert x.shape == (B, N, F_IN) and adj.shape == (B, N, N)
    nc = get_nc(MODE)
    in_maps = make_in_maps(x, adj, W, b)
    res = run_bass_kernel_spmd(nc, in_maps, core_ids=list(range(B)))
    return np.stack([res.results[k]["out"] for k in range(B)]).astype(np.float32)
